# revision 20
# baseline (speedup 1.0000x reference)
"""Trainium2 Bass kernel for nn_Block_65386582114509 (dense transformer block).

Strategy: pure data parallelism — B=8 batch elements, one per NeuronCore.
Per-core dataflow keeps activations TRANSPOSED ([feature, seq] layout) so
every matmul contracts over the partition dim with weights stationary:

  qkv:    qT/kT = W.T @ xT            (weight stationary, xT moving)
  V:      V natural = x @ Wv          (xT stationary, Wv moving)
  scores: S^T = kT-slice.T @ qT       ([sk, sq] layout, softmax-friendly)
  exp:    ACT Exp(scale=1/8, bias=padding-mask column); causal via mask mul
  PV:     [V|1].T @ expS^T            -> unnormalized ctx^T + denominator row
  merge:  0.5/denom broadcast via K=1 matmul, ctx averaged self/cross
  oproj:  Wo.T @ ctxT (+x residual on host-pre-biased xT)
  LN:     transposed layernorm — mean/var via ones-matmul (f32r), broadcast
          of A=rstd, B=-mu*rstd via K=1 matmul, per-partition affine g/b
  MLP:    fcT = Wfc.T @ nT -> gelu -> mT; mprojT = Wm.T @ mT; residual; LN2

No max-subtraction in softmax: scores*1/8 are O(3), masked lanes get
exp(-1e5)=0 exactly, matching the reference numerics within tolerance.
"""

import numpy as np
import ml_dtypes

import concourse.bass as bass
import concourse.tile as tile
from concourse import mybir
from concourse.bass_utils import run_bass_kernel_spmd
from contextlib import ExitStack

B, S, SE, D, H, HD = 8, 1024, 512, 1024, 16, 64
P = 128            # partitions
KT = D // P        # 8 k-tiles over the model dim
NCH = S // 512     # 2 seq chunks of 512
NEG = -100000.0
EPS = 1e-5

F32 = mybir.dt.float32
F32R = mybir.dt.float32r
BF16 = mybir.dt.bfloat16
AF = mybir.ActivationFunctionType
OP = mybir.AluOpType


def _split_sync_waits(nc, max_waits=1):
    """walrus in this container rejects >max_waits semaphore waits per
    instruction; split the excess onto preceding same-engine no-ops
    (engine queues are in-order, so the gating is preserved)."""
    n_split = 0
    for fn in nc.m.functions:
        for bb in fn.blocks:
            idx = 0
            while idx < len(bb.instructions):
                inst = bb.instructions[idx]
                si = inst.sync_info
                if si is not None and si.on_wait and len(si.on_wait) > max_waits:
                    waits = list(si.on_wait)
                    keep = waits[-max_waits:]
                    excess = waits[:-max_waits]
                    nops = []
                    for i in range(0, len(excess), max_waits):
                        chunk = excess[i:i + max_waits]
                        nop = mybir.InstNoOp(
                            name=f"waitsplit_{n_split}", ins=[], outs=[])
                        nop.engine = inst.engine
                        nop.sync_info = mybir.SyncInfo(on_wait=chunk, on_update=[])
                        nops.append(nop)
                        n_split += 1
                    inst.sync_info = mybir.SyncInfo(
                        on_wait=keep, on_update=list(si.on_update or []))
                    for j, nop in enumerate(nops):
                        bb.instructions.insert(idx + j, nop)
                    idx += len(nops)
                idx += 1
    return n_split





def _emit(nc, tc, ctx, d):
    const = ctx.enter_context(tc.tile_pool(name="const", bufs=1))

    def col_load(name, n):
        t = const.tile([P, n], F32, tag=name)
        nc.sync.dma_start(t[:], d[name].ap().rearrange("(m p) -> p m", p=P))
        return t

    bqk = col_load("b_qk", 16)
    bfc = col_load("b_fc", 32)
    bm = col_load("b_m", 8)
    l1g = col_load("ln1_g", 8)
    l1b = col_load("ln1_b", 8)
    l2g = col_load("ln2_g", 8)
    l2b = col_load("ln2_b", 8)
    amask = col_load("amask", 4)

    bvb = const.tile([P, D], F32)  # b_v broadcast along partitions
    bv_ap = d["b_v"].ap()
    nc.sync.dma_start(
        bvb[:], bass.AP(tensor=bv_ap.tensor, offset=bv_ap.offset,
                        ap=[[0, P]] + list(bv_ap.ap)))
    cmask = const.tile([P, 4, 512], BF16)
    nc.sync.dma_start(cmask[:], d["cmask"].ap().rearrange(
        "p (k f) -> p k f", f=512))

    inv_ones = const.tile([P, 1], BF16, tag="inv_ones")
    nc.vector.memset(inv_ones[:], 1.0 / D)
    ones_b = const.tile([1, P], F32, tag="ones_b")
    nc.vector.memset(ones_b[:], 1.0)
    halves = const.tile([1, 64], BF16, tag="halves")
    nc.vector.memset(halves[:], 0.5)
    eps_t = const.tile([1, 1], F32, tag="eps")
    nc.vector.memset(eps_t[:], EPS)

    # ---------------- persistent attention operands ----------------
    patt = ExitStack()
    qT = patt.enter_context(tc.tile_pool(name="qT", bufs=1)).tile([P, KT, S], BF16)
    kT = patt.enter_context(tc.tile_pool(name="kT", bufs=1)).tile([P, KT, S], BF16)
    kTc = patt.enter_context(tc.tile_pool(name="kTc", bufs=1)).tile([P, KT, SE], BF16)
    vaug = patt.enter_context(tc.tile_pool(name="vaug", bufs=1)).tile(
        [P, KT, H, HD + 1], BF16)
    vcaug = patt.enter_context(tc.tile_pool(name="vcaug", bufs=1)).tile(
        [P, SE // P, H, HD + 1], BF16)

    # ---------------- phase A: qkv / cross-kv / V ----------------
    with ExitStack() as pa:
        big = pa.enter_context(tc.tile_pool(name="phA_big", bufs=1))
        xT = big.tile([P, KT, S], BF16)
        nc.sync.dma_start(xT[:], d["xT_bf"].ap().rearrange("(kt p) s -> p kt s", p=P))
        encT = big.tile([P, KT, SE], BF16)
        nc.sync.dma_start(encT[:], d["encT_bf"].ap().rearrange(
            "(kt p) s -> p kt s", p=P))
        wqk = big.tile([P, KT, 2 * D], BF16)
        nc.sync.dma_start(wqk[:], d["w_qk"].ap().rearrange("(kt p) n -> p kt n", p=P))
        wv = big.tile([P, KT, D], BF16)
        nc.sync.dma_start(wv[:], d["w_v"].ap().rearrange("(kt p) n -> p kt n", p=P))

        psA = pa.enter_context(tc.tile_pool(name="phA_psum", bufs=4, space="PSUM"))

        # qT / kT: weight-stationary, xT moving.  m 0..7 -> q, 8..15 -> k
        for m in range(16):
            dst = qT if m < 8 else kT
            ml = m % 8
            for c in range(NCH):
                ps = psA.tile([P, 512], F32)
                for k in range(KT):
                    nc.tensor.matmul(
                        ps[:], wqk[:, k, m * P:(m + 1) * P],
                        xT[:, k, c * 512:(c + 1) * 512],
                        start=(k == 0), stop=(k == KT - 1))
                nc.scalar.activation(dst[:, ml, c * 512:(c + 1) * 512], ps[:],
                                     AF.Identity, bias=bqk[:, m:m + 1])
        # cross kT: enc moving, w_k stationary (w_qk cols 1024..2047)
        for m in range(8):
            ps = psA.tile([P, 512], F32)
            for k in range(KT):
                nc.tensor.matmul(
                    ps[:], wqk[:, k, D + m * P:D + (m + 1) * P],
                    encT[:, k, :],
                    start=(k == 0), stop=(k == KT - 1))
            nc.scalar.activation(kTc[:, m, :], ps[:],
                                 AF.Identity, bias=bqk[:, 8 + m:9 + m])
        # V natural: xT stationary, w_v moving -> [s, dv]
        for st in range(KT):
            for half in range(2):
                ps = psA.tile([P, 512], F32)
                for k in range(KT):
                    nc.tensor.matmul(
                        ps[:], xT[:, k, st * P:(st + 1) * P],
                        wv[:, k, half * 512:(half + 1) * 512],
                        start=(k == 0), stop=(k == KT - 1))
                nc.vector.tensor_add(
                    vaug[:, st, half * 8:(half + 1) * 8, 0:HD],
                    ps[:].rearrange("p (h d) -> p h d", d=HD),
                    bvb[:, half * 512:(half + 1) * 512].rearrange(
                        "p (h d) -> p h d", d=HD))
        # V cross natural: encT stationary
        for st in range(SE // P):
            for half in range(2):
                ps = psA.tile([P, 512], F32)
                for k in range(KT):
                    nc.tensor.matmul(
                        ps[:], encT[:, k, st * P:(st + 1) * P],
                        wv[:, k, half * 512:(half + 1) * 512],
                        start=(k == 0), stop=(k == KT - 1))
                nc.vector.tensor_add(
                    vcaug[:, st, half * 8:(half + 1) * 8, 0:HD],
                    ps[:].rearrange("p (h d) -> p h d", d=HD),
                    bvb[:, half * 512:(half + 1) * 512].rearrange(
                        "p (h d) -> p h d", d=HD))
        nc.vector.memset(vaug[:, :, :, HD:HD + 1], 1.0)
        nc.vector.memset(vcaug[:, :, :, HD:HD + 1], 1.0)

    # aavg/xf32 span attention..oproj, overlapping both the attention pools
    # and the nT pools; they live on the RIGHT allocation stack so each
    # side's pool lifetime stays LIFO.
    pab = ExitStack()
    aavg = pab.enter_context(
        tc.tile_pool(name="aavg", bufs=1, side="right")).tile([P, KT, S], BF16)
    xf32 = pab.enter_context(
        tc.tile_pool(name="xf32", bufs=1, side="right")).tile([P, KT, S], F32)
    nc.sync.dma_start(xf32[:], d["xT_f32b"].ap().rearrange(
        "(kt p) s -> p kt s", p=P))

    # ---------------- attention ----------------
    with ExitStack() as pat:
        expp = pat.enter_context(tc.tile_pool(name="expS", bufs=16))
        sps = pat.enter_context(tc.tile_pool(name="score_ps", bufs=2, space="PSUM"))
        aps = pat.enter_context(tc.tile_pool(name="acc_ps", bufs=2, space="PSUM"))
        bps = pat.enter_context(tc.tile_pool(name="bcast_ps", bufs=2, space="PSUM"))
        rows = pat.enter_context(tc.tile_pool(name="rows", bufs=4))
        mrg = pat.enter_context(tc.tile_pool(name="mrg", bufs=4))

        for h in range(H):
            p0 = (h % 2) * 64
            mq = h // 2
            for c in range(NCH):
                qh = qT[p0:p0 + 64, mq, c * 512:(c + 1) * 512]
                # -- self scores + exp (causal) --
                nsk = 4 * (c + 1)
                exps = []
                for sk in range(nsk):
                    ps = sps.tile([P, 512], F32, tag="sc")
                    nc.tensor.matmul(
                        ps[:], kT[p0:p0 + 64, mq, sk * P:(sk + 1) * P], qh,
                        start=True, stop=True)
                    e = expp.tile([P, 512], BF16, tag="e")
                    nc.scalar.activation(e[:], ps[:], AF.Exp, scale=0.125)
                    delta = sk * P - c * 512
                    if delta >= 0:  # diagonal-crossing tile: causal mask
                        nc.vector.tensor_mul(
                            e[:], e[:], cmask[:, delta // P, :])
                    exps.append(e)
                # -- self PV --
                acc_s = aps.tile([HD + 1, 512], F32, tag="acc")
                for i, e in enumerate(exps):
                    nc.tensor.matmul(acc_s[:], vaug[:, i, h, :], e[:],
                                     start=(i == 0), stop=(i == len(exps) - 1))
                # -- cross scores + exp (padding mask as exp bias) --
                expc = []
                for sk in range(SE // P):
                    ps = sps.tile([P, 512], F32, tag="sc")
                    nc.tensor.matmul(
                        ps[:], kTc[p0:p0 + 64, mq, sk * P:(sk + 1) * P], qh,
                        start=True, stop=True)
                    e = expp.tile([P, 512], BF16, tag="e")
                    nc.scalar.activation(e[:], ps[:], AF.Exp, scale=0.125,
                                         bias=amask[:, sk:sk + 1])
                    expc.append(e)
                acc_c = aps.tile([HD + 1, 512], F32, tag="acc")
                for i, e in enumerate(expc):
                    nc.tensor.matmul(acc_c[:], vcaug[:, i, h, :], e[:],
                                     start=(i == 0), stop=(i == len(expc) - 1))
                # -- merge: 0.5/denom broadcast + weighted average --
                rs = rows.tile([1, 512], BF16, tag="row")
                rc = rows.tile([1, 512], BF16, tag="row")
                with nc.allow_low_precision("softmax recip rows feed bf16 bcast"):
                    nc.vector.reciprocal(rs[:], acc_s[64:65, :])
                    nc.vector.reciprocal(rc[:], acc_c[64:65, :])
                rs_b = bps.tile([64, 512], F32, tag="bc")
                nc.tensor.matmul(rs_b[:], halves[:], rs[:],
                                 start=True, stop=True)
                rc_b = bps.tile([64, 512], F32, tag="bc")
                nc.tensor.matmul(rc_b[:], halves[:], rc[:],
                                 start=True, stop=True)
                rs_sb = mrg.tile([64, 512], F32, tag="rsb")
                nc.scalar.copy(rs_sb[:], rs_b[:])
                rc_sb = mrg.tile([64, 512], F32, tag="rsb")
                nc.scalar.copy(rc_sb[:], rc_b[:])
                t1 = mrg.tile([64, 512], F32, tag="t")
                nc.vector.tensor_mul(t1[:], acc_s[0:64, :], rs_sb[:])
                t2 = mrg.tile([64, 512], F32, tag="t")
                nc.vector.tensor_mul(t2[:], acc_c[0:64, :], rc_sb[:])
                nc.vector.tensor_add(
                    aavg[p0:p0 + 64, mq, c * 512:(c + 1) * 512], t1[:], t2[:])

    patt.close()  # free qT/kT/kTc/V

    # ---------------- oproj + LN1 ----------------
    pbn = ExitStack()  # nT spans oproj..mproj
    nT = pbn.enter_context(tc.tile_pool(name="nT", bufs=1)).tile([P, KT, S], F32)
    nTb = pbn.enter_context(tc.tile_pool(name="nTb", bufs=1)).tile([P, KT, S], BF16)

    ln_ctr = [0]

    def ln_rows_and_apply(r1, c, g, b, out_f32, out_bf):
        """transposed layernorm over the 8 partition tiles of chunk c.
        r1: [P, KT, 512] f32 AP (chunk slice); writes out_f32/out_bf slices."""
        ln_ctr[0] += 1
        with ExitStack() as pl:
            lps = pl.enter_context(tc.tile_pool(
                name=f"ln_ps{ln_ctr[0]}", bufs=1, space="PSUM"))
            lsb = pl.enter_context(tc.tile_pool(name=f"ln_sb{ln_ctr[0]}", bufs=2))
            lrow = pl.enter_context(tc.tile_pool(name=f"ln_row{ln_ctr[0]}", bufs=1))
            mu_ps = lps.tile([1, 512], F32, tag="mu")
            e2_ps = lps.tile([1, 512], F32, tag="e2")
            for k in range(KT):
                r1b = lsb.tile([P, 512], BF16, tag="r1b")
                nc.vector.tensor_copy(r1b[:], r1[:, k, :])
                sq = lsb.tile([P, 512], BF16, tag="sq")
                nc.vector.tensor_mul(sq[:], r1b[:], r1b[:])
                nc.tensor.matmul(mu_ps[:], inv_ones[:], r1b[:],
                                 start=(k == 0), stop=(k == KT - 1))
                nc.tensor.matmul(e2_ps[:], inv_ones[:], sq[:],
                                 start=(k == 0), stop=(k == KT - 1))
            mu = lrow.tile([1, 512], F32, tag="mu")
            nc.scalar.copy(mu[:], mu_ps[:])
            mu2 = lrow.tile([1, 512], F32, tag="mu2")
            nc.vector.tensor_mul(mu2[:], mu[:], mu[:])
            var = lrow.tile([1, 512], F32, tag="var")
            nc.vector.scalar_tensor_tensor(var[:], e2_ps[:], 1.0, mu2[:],
                                           OP.mult, OP.subtract)
            sd = lrow.tile([1, 512], F32, tag="sd")
            nc.scalar.activation(sd[:], var[:], AF.Sqrt, bias=eps_t[:])
            rstd = lrow.tile([1, 512], F32, tag="rstd")
            nc.vector.reciprocal(rstd[:], sd[:])
            bb = lrow.tile([1, 512], F32, tag="bb")
            nc.vector.scalar_tensor_tensor(bb[:], mu[:], -1.0, rstd[:],
                                           OP.mult, OP.mult)
            a_b = lps.tile([P, 512], F32, tag="bca")
            nc.tensor.matmul(a_b[:], ones_b[:], rstd[:],
                             start=True, stop=True)
            b_b = lps.tile([P, 512], F32, tag="bcb")
            nc.tensor.matmul(b_b[:], ones_b[:], bb[:],
                             start=True, stop=True)
            for k in range(KT):
                t = lsb.tile([P, 512], F32, tag="t")
                nc.vector.tensor_mul(t[:], r1[:, k, :], a_b[:])
                t2 = lsb.tile([P, 512], F32, tag="t")
                nc.vector.tensor_add(t2[:], t[:], b_b[:])
                nc.vector.tensor_scalar(
                    out_f32[:, k, :], t2[:], g[:, k:k + 1], b[:, k:k + 1],
                    OP.mult, OP.add)
                if out_bf is not None:
                    nc.scalar.copy(out_bf[:, k, :], out_f32[:, k, :])

    with ExitStack() as pb:
        wo = pb.enter_context(tc.tile_pool(name="wo", bufs=1)).tile(
            [P, KT, D], BF16)
        nc.sync.dma_start(wo[:], d["w_o"].ap().rearrange("(kt p) n -> p kt n", p=P))
        r1 = pb.enter_context(tc.tile_pool(name="r1", bufs=1)).tile([P, KT, S], F32)
        ops = pb.enter_context(tc.tile_pool(name="op_ps", bufs=3, space="PSUM"))
        for c in range(NCH):
            for m in range(KT):
                ps = ops.tile([P, 512], F32)
                for k in range(KT):
                    nc.tensor.matmul(
                        ps[:], wo[:, k, m * P:(m + 1) * P],
                        aavg[:, k, c * 512:(c + 1) * 512],
                        start=(k == 0), stop=(k == KT - 1))
                nc.vector.tensor_add(r1[:, m, c * 512:(c + 1) * 512], ps[:],
                                     xf32[:, m, c * 512:(c + 1) * 512])
            ln_rows_and_apply(r1[:, :, c * 512:(c + 1) * 512], c, l1g, l1b,
                              nT[:, :, c * 512:(c + 1) * 512],
                              nTb[:, :, c * 512:(c + 1) * 512])

    pab.close()  # free aavg, xf32

    # ---------------- MLP + LN2 ----------------
    with ExitStack() as pc:
        wmp = pc.enter_context(tc.tile_pool(name="wm", bufs=2))
        wfcp = pc.enter_context(tc.tile_pool(name="wfc", bufs=2))
        mT = pc.enter_context(tc.tile_pool(name="mT", bufs=1)).tile(
            [P, 32, 512], BF16)
        r2 = pc.enter_context(tc.tile_pool(name="r2", bufs=1)).tile(
            [P, KT, 512], F32)
        h_out = pc.enter_context(tc.tile_pool(name="h_out", bufs=1)).tile(
            [P, KT, 512], F32)
        mps = pc.enter_context(tc.tile_pool(name="mlp_ps", bufs=3, space="PSUM"))

        for c in range(NCH):
            # fc + gelu -> mT (chunk c)
            for g in range(8):  # 8 groups of 4 hidden m-tiles
                wfc = wfcp.tile([P, KT, 512], BF16, tag="wfc")
                nc.sync.dma_start(
                    wfc[:], d["w_fc"].ap()[:, g * 512:(g + 1) * 512].rearrange(
                        "(kt p) n -> p kt n", p=P))
                for ml in range(4):
                    m = g * 4 + ml
                    ps = mps.tile([P, 512], F32)
                    for k in range(KT):
                        nc.tensor.matmul(
                            ps[:], wfc[:, k, ml * P:(ml + 1) * P],
                            nTb[:, k, c * 512:(c + 1) * 512],
                            start=(k == 0), stop=(k == KT - 1))
                    nc.scalar.activation(mT[:, m, :], ps[:], AF.Gelu_apprx_tanh,
                                         bias=bfc[:, m:m + 1])
            # mproj -> r2 = psum + b_m + n  (w_m streamed per output tile)
            for m2 in range(KT):
                wm = wmp.tile([P, 32, P], BF16, tag="wm")
                nc.sync.dma_start(
                    wm[:], d["w_m"].ap()[:, m2 * P:(m2 + 1) * P].rearrange(
                        "(kt p) n -> p kt n", p=P))
                ps = mps.tile([P, 512], F32)
                for k in range(32):
                    nc.tensor.matmul(
                        ps[:], wm[:, k, :], mT[:, k, :],
                        start=(k == 0), stop=(k == 31))
                nc.vector.scalar_tensor_tensor(
                    r2[:, m2, :], ps[:], bm[:, m2:m2 + 1],
                    nT[:, m2, c * 512:(c + 1) * 512], OP.add, OP.add)
            ln_rows_and_apply(r2[:], c, l2g, l2b, h_out[:], None)
            for m2 in range(KT):
                nc.sync.dma_start(
                    d["hT"].ap()[m2 * P:(m2 + 1) * P, c * 512:(c + 1) * 512],
                    h_out[:, m2, :])

    pbn.close()


def build_kernel():
    nc = bass.Bass()
    d = {}
    def di(name, shape, dt):
        d[name] = nc.dram_tensor(name, shape, dt, kind="ExternalInput")
    di("xT_bf", [D, S], BF16)
    di("xT_f32b", [D, S], F32)
    di("encT_bf", [D, SE], BF16)
    di("w_qk", [D, 2 * D], BF16)
    di("w_v", [D, D], BF16)
    di("w_o", [D, D], BF16)
    di("w_fc", [D, 4 * D], BF16)
    di("w_m", [4 * D, D], BF16)
    di("b_qk", [2 * D], F32)
    di("b_v", [D], F32)
    di("b_fc", [4 * D], F32)
    di("b_m", [D], F32)
    di("ln1_g", [D], F32)
    di("ln1_b", [D], F32)
    di("ln2_g", [D], F32)
    di("ln2_b", [D], F32)
    di("amask", [SE], F32)
    di("cmask", [P, 2048], BF16)
    d["hT"] = nc.dram_tensor("hT", [D, S], F32, kind="ExternalOutput")

    with tile.TileContext(nc) as tc, ExitStack() as ctx:
        _emit(nc, tc, ctx, d)
    _split_sync_waits(nc, 1)
    return nc


_NC_CACHE = {}


def _get_nc():
    if "nc" not in _NC_CACHE:
        _NC_CACHE["nc"] = build_kernel()
    return _NC_CACHE["nc"]


def _host_prep(x, enc_hidden, enc_padding_mask, w_attn, b_attn, w_oproj,
               b_oproj, w_fc, b_fc, w_mproj, b_mproj,
               ln1_g, ln1_b, ln2_g, ln2_b):
    bf = ml_dtypes.bfloat16
    f32 = np.float32
    x = np.asarray(x, f32)
    enc = np.asarray(enc_hidden, f32)
    pad = np.asarray(enc_padding_mask)
    w_attn = np.asarray(w_attn, f32)
    b_attn = np.asarray(b_attn, f32)
    w_oproj = np.asarray(w_oproj, f32)
    b_oproj = np.asarray(b_oproj, f32)

    shared = {
        "w_qk": np.ascontiguousarray(w_attn[:, :2 * D]).astype(bf),
        "w_v": np.ascontiguousarray(w_attn[:, 2 * D:]).astype(bf),
        "w_o": w_oproj.astype(bf),
        "w_fc": np.asarray(w_fc, f32).astype(bf),
        "w_m": np.asarray(w_mproj, f32).astype(bf),
        "b_qk": np.ascontiguousarray(b_attn[:2 * D]),
        "b_v": np.ascontiguousarray(b_attn[2 * D:]),
        "b_fc": np.asarray(b_fc, f32),
        "b_m": np.asarray(b_mproj, f32),
        "ln1_g": np.asarray(ln1_g, f32), "ln1_b": np.asarray(ln1_b, f32),
        "ln2_g": np.asarray(ln2_g, f32), "ln2_b": np.asarray(ln2_b, f32),
    }
    pp = np.arange(P)[:, None]
    ff = np.arange(512)[None, :]
    cm = np.stack([(ff - pp >= 128 * k) for k in range(4)], axis=1)
    shared["cmask"] = cm.reshape(P, 2048).astype(bf)

    in_maps = []
    for b in range(B):
        xT = np.ascontiguousarray(x[b].T)
        m = dict(shared)
        m["xT_bf"] = xT.astype(bf)
        m["xT_f32b"] = np.ascontiguousarray(xT + b_oproj[:, None])
        m["encT_bf"] = np.ascontiguousarray(enc[b].T).astype(bf)
        m["amask"] = (pad[b].astype(f32) * NEG)
        in_maps.append(m)
    return in_maps


def kernel(x, enc_hidden, enc_padding_mask, w_attn, b_attn, w_oproj, b_oproj,
           w_fc, b_fc, w_mproj, b_mproj, ln1_g, ln1_b, ln2_g, ln2_b):
    nc = _get_nc()
    in_maps = _host_prep(x, enc_hidden, enc_padding_mask, w_attn, b_attn,
                         w_oproj, b_oproj, w_fc, b_fc, w_mproj, b_mproj,
                         ln1_g, ln1_b, ln2_g, ln2_b)
    res = run_bass_kernel_spmd(nc, in_maps, list(range(B)))
    out = np.stack([np.ascontiguousarray(res.results[b]["hT"].T)
                    for b in range(B)]).astype(np.float32)
    return out
